# revision 19
# baseline (speedup 1.0000x reference)
"""Bass/Trainium2 kernel for nn_MultiHeadedAttention (GQA + RoPE + causal attention).

Sharding: 8 cores = 2 batch groups x 4 head-groups.
Core c: batch b=c//4, head group j=c%4 (q heads 4j..4j+3, kv head j).
Output projection is column-sharded after a 4-way AllGather of ctx^T;
host concatenates the disjoint output slices.

Compute is bf16 on the TensorEngine (fp32 PSUM accumulation), which
enables fast weight loads and halves DMA/collective traffic.
"""

import os
import sys

sys.path.insert(0, "/opt/trn_rl_repo")
import numpy as np


B, S, HID = 2, 2048, 2048
NH, NKV, D = 16, 4, 128
N_CORES = 8
GROUPS = [[0, 1, 2, 3], [4, 5, 6, 7]]
HLOC = 4          # q heads per core
TB = 512          # token block (matmul moving dim)
NTB = S // TB     # 4
HT = HID // 128   # 16 hid tiles
SCALE = float(D) ** -0.5

LAST_RESULTS = None  # stash for test harness timing


def _analyze_mask(mask):
    """Per (qblock, ktile): live pairs and mixed-mask tiles (deduped)."""
    maskb = np.asarray(mask).astype(bool)
    live = []
    mixd = {}
    uniq = []
    keys = {}
    for qb in range(NTB):
        lv = []
        for kt in range(S // 128):
            sub = maskb[qb * TB:(qb + 1) * TB, kt * 128:(kt + 1) * 128]
            if not sub.any():
                continue
            lv.append(kt)
            if sub.all():
                mixd[(qb, kt)] = None
            else:
                tile = np.ascontiguousarray(sub.T.astype(np.float32))
                kb = tile.tobytes()
                if kb not in keys:
                    keys[kb] = len(uniq)
                    uniq.append(tile)
                mixd[(qb, kt)] = keys[kb]
        live.append(lv)
    return live, mixd, uniq


def _build_program(live, mixd, n_u):
    import concourse.bass as bass  # noqa: F401
    import concourse.mybir as mybir
    from concourse import bacc, tile

    f32 = mybir.dt.float32
    bf16 = mybir.dt.bfloat16
    EXP = mybir.ActivationFunctionType.Exp

    nc = bacc.Bacc("TRN2", target_bir_lowering=False, debug=False,
                   num_devices=N_CORES)

    xT = nc.dram_tensor("xT", [HID, S], bf16, kind="ExternalInput")
    wq = nc.dram_tensor("wq", [HID, HLOC * D], bf16, kind="ExternalInput")
    wk = nc.dram_tensor("wk", [HID, D], bf16, kind="ExternalInput")
    wv = nc.dram_tensor("wv", [HID, D], bf16, kind="ExternalInput")
    wo = nc.dram_tensor("wo", [HID, TB], bf16, kind="ExternalInput")
    cosE = nc.dram_tensor("cosE", [D, S], bf16, kind="ExternalInput")
    sinP = nc.dram_tensor("sinP", [D, S], bf16, kind="ExternalInput")
    pswap = nc.dram_tensor("pswap", [128, 128], bf16, kind="ExternalInput")
    ident = nc.dram_tensor("ident", [128, 128], bf16, kind="ExternalInput")
    ones_in = nc.dram_tensor("ones_in", [128, 1], bf16, kind="ExternalInput")
    onesk1_in = nc.dram_tensor("onesk1_in", [1, 128], bf16, kind="ExternalInput")
    mmask = nc.dram_tensor("mmask", [max(n_u, 1) * 128, TB], bf16,
                           kind="ExternalInput")
    out_o = nc.dram_tensor("o", [S, TB], f32, kind="ExternalOutput")

    mm = nc.tensor.matmul

    with tile.TileContext(nc, num_cores=N_CORES) as tc:
        stk0 = nc.allow_low_precision("bf16 kernel; fp32 PSUM accumulate")
        stk0.__enter__()
        with (
            tc.tile_pool(name="const", bufs=1) as cpool,
            tc.tile_pool(name="acts", bufs=1) as apool,
            tc.tile_pool(name="dram", bufs=1, space="DRAM") as dram,
        ):
            ones_s = cpool.tile([128, 1], bf16, tag="ones")
            nc.sync.dma_start(out=ones_s[:], in_=ones_in[:])
            onesk1 = cpool.tile([1, 128], bf16, tag="onesk1")
            nc.sync.dma_start(out=onesk1[:], in_=onesk1_in[:])
            mm_s = None
            if n_u:
                mm_s = cpool.tile([128, n_u * TB], bf16, tag="mm")
                nc.sync.dma_start(
                    out=mm_s[:].rearrange("p (u n) -> p u n", n=TB),
                    in_=mmask[:].rearrange("(u p) n -> p u n", p=128),
                )

            qT_s = apool.tile([128, HLOC * S], bf16, tag="qT")
            kT_s = apool.tile([128, S], bf16, tag="kT")
            v_s = apool.tile([128, S], bf16, tag="v")
            ctxT_s = apool.tile([128, HLOC * S], bf16, tag="ctxT")
            wo_s = apool.tile([128, HT * TB], bf16, tag="wo")

            bounce = [dram.tile([128, S], bf16, tag=f"bn{h}", name=f"bounce{h}")
                      for h in range(HLOC)]
            gath = [dram.tile([HLOC * 128, S], bf16, tag=f"g{h}", name=f"gath{h}")
                    for h in range(HLOC)]

            # ---------- Phase 1: QKV projections + RoPE + V transpose ----------
            with (
                tc.tile_pool(name="w1", bufs=1) as wpool,
                tc.tile_pool(name="xs", bufs=4) as xpool,
                tc.tile_pool(name="p1", bufs=1, space="PSUM") as p1,
                tc.tile_pool(name="p1b", bufs=2, space="PSUM") as p1b,
                tc.tile_pool(name="st1", bufs=2) as stage,
            ):
                ps_s = wpool.tile([128, 128], bf16, tag="ps")
                nc.sync.dma_start(out=ps_s[:], in_=pswap[:])
                id_s = wpool.tile([128, 128], bf16, tag="id")
                nc.sync.dma_start(out=id_s[:], in_=ident[:])
                cos_s = wpool.tile([D, S], bf16, tag="cos")
                nc.sync.dma_start(out=cos_s[:], in_=cosE[:])
                sin_s = wpool.tile([D, S], bf16, tag="sin")
                nc.sync.dma_start(out=sin_s[:], in_=sinP[:])
                wq_s = wpool.tile([128, HT * HLOC * D], bf16, tag="wq")
                nc.sync.dma_start(
                    out=wq_s[:].rearrange("p (h n) -> p h n", n=HLOC * D),
                    in_=wq[:].rearrange("(h p) n -> p h n", p=128),
                )
                wk_s = wpool.tile([128, HT * D], bf16, tag="wk")
                nc.sync.dma_start(
                    out=wk_s[:].rearrange("p (h n) -> p h n", n=D),
                    in_=wk[:].rearrange("(h p) n -> p h n", p=128),
                )
                wv_s = wpool.tile([128, HT * D], bf16, tag="wv")
                nc.sync.dma_start(
                    out=wv_s[:].rearrange("p (h n) -> p h n", n=D),
                    in_=wv[:].rearrange("(h p) n -> p h n", p=128),
                )

                for t in range(NTB):
                    qps = [p1.tile([128, TB], f32, tag=f"qps{i}", name=f"qps{i}")
                           for i in range(HLOC)]
                    kps = p1.tile([128, TB], f32, tag="kps")
                    vps = p1.tile([128, TB], f32, tag="vps")
                    for h in range(HT):
                        xt = xpool.tile([128, TB], bf16, tag="xt")
                        nc.sync.dma_start(
                            out=xt[:],
                            in_=xT[h * 128:(h + 1) * 128, t * TB:(t + 1) * TB],
                        )
                        st, sp = (h == 0), (h == HT - 1)
                        for i in range(HLOC):
                            mm(qps[i][:],
                               wq_s[:, h * HLOC * D + i * D: h * HLOC * D + (i + 1) * D],
                               xt[:], start=st, stop=sp)
                        mm(kps[:], wk_s[:, h * D:(h + 1) * D], xt[:], start=st, stop=sp)
                        mm(vps[:], wv_s[:, h * D:(h + 1) * D], xt[:], start=st, stop=sp)

                    # V: transpose [tok, d] tiles into v_s
                    vstg = stage.tile([128, TB], bf16, tag="vstg")
                    nc.vector.tensor_copy(vstg[:], vps[:])
                    for i in range(TB // 128):
                        tps = p1b.tile([128, 128], bf16, tag="aux", name="tps")
                        nc.tensor.transpose(tps[:], vstg[:, i * 128:(i + 1) * 128], id_s[:])
                        tt = t * (TB // 128) + i
                        nc.vector.tensor_copy(v_s[:, tt * 128:(tt + 1) * 128], tps[:])

                    # Q/K: copy to SBUF, then RoPE in place
                    chunks = []
                    for i in range(HLOC):
                        qc = qT_s[:, i * S + t * TB: i * S + (t + 1) * TB]
                        nc.vector.tensor_copy(qc, qps[i][:])
                        chunks.append(qc)
                    kc = kT_s[:, t * TB:(t + 1) * TB]
                    nc.vector.tensor_copy(kc, kps[:])
                    chunks.append(kc)

                    for ch in chunks:
                        sw = p1b.tile([128, TB], f32, tag="aux", name="swps")
                        mm(sw[:], ps_s[:], ch, start=True, stop=True)
                        swm = stage.tile([128, TB], bf16, tag="swm")
                        nc.vector.tensor_mul(swm[:], sw[:], sin_s[:, t * TB:(t + 1) * TB])
                        nc.vector.tensor_mul(ch, ch, cos_s[:, t * TB:(t + 1) * TB])
                        nc.vector.tensor_add(ch, ch, swm[:])

                nc.sync.dma_start(
                    out=wo_s[:].rearrange("p (h n) -> p h n", n=TB),
                    in_=wo[:].rearrange("(h p) n -> p h n", p=128),
                )

            # ---------- Phase 2: attention per head, AllGather per head ----------
            with (
                tc.tile_pool(name="ex", bufs=6) as epool,
                tc.tile_pool(name="bc", bufs=2) as bcpool,
                tc.tile_pool(name="rc", bufs=2) as rcpool,
                tc.tile_pool(name="p2s", bufs=3, space="PSUM") as p2s,
                tc.tile_pool(name="p2c", bufs=2, space="PSUM") as p2c,
                tc.tile_pool(name="p2d", bufs=2, space="PSUM") as p2d,
                tc.tile_pool(name="p2b", bufs=1, space="PSUM") as p2b,
            ):
                for h in range(HLOC):
                    for qb in range(NTB):
                        lv = live[qb]
                        cps = p2c.tile([128, TB], f32, tag="cps")
                        dps = p2d.tile([1, TB], f32, tag="dps")
                        qslice = qT_s[:, h * S + qb * TB: h * S + (qb + 1) * TB]
                        npair = (len(lv) + 1) // 2
                        prev_ex = None
                        for idx, kt in enumerate(lv):
                            st, sp = (idx == 0), (idx == len(lv) - 1)
                            sps = p2s.tile([128, TB], f32, tag="sps")
                            mm(sps[:], kT_s[:, kt * 128:(kt + 1) * 128], qslice,
                               start=True, stop=True)
                            ex = epool.tile([128, TB], bf16, tag="ex")
                            nc.scalar.activation(ex[:], sps[:], EXP, scale=SCALE)
                            u = mixd[(qb, kt)]
                            if u is not None:
                                nc.vector.tensor_mul(ex[:], ex[:],
                                                     mm_s[:, u * TB:(u + 1) * TB])
                            mm(cps[:], v_s[:, kt * 128:(kt + 1) * 128], ex[:],
                               start=st, stop=sp)
                            # denominator: pairwise pre-sum on DVE, ones-matmul per pair
                            pi = idx // 2
                            pst, psp = (pi == 0), (pi == npair - 1)
                            if idx % 2 == 0:
                                if sp:  # odd tail: single tile closes the pair group
                                    mm(dps[:], ones_s[:], ex[:], start=pst, stop=psp)
                                else:
                                    prev_ex = ex
                            else:
                                exs = epool.tile([128, TB], bf16, tag="exs")
                                nc.vector.tensor_add(exs[:], prev_ex[:], ex[:])
                                mm(dps[:], ones_s[:], exs[:], start=pst, stop=psp)
                        rc = rcpool.tile([1, TB], f32, tag="rc")
                        nc.vector.reciprocal_approx_fast(rc[:], dps[:])
                        rcb = rcpool.tile([1, TB], bf16, tag="rcb")
                        nc.vector.tensor_copy(rcb[:], rc[:])
                        bps = p2b.tile([128, TB], f32, tag="bps")
                        mm(bps[:], onesk1[:], rcb[:], start=True, stop=True)
                        bcs = bcpool.tile([128, TB], bf16, tag="bcs")
                        nc.vector.tensor_copy(bcs[:], bps[:])
                        nc.vector.tensor_mul(
                            ctxT_s[:, h * S + qb * TB: h * S + (qb + 1) * TB],
                            cps[:], bcs[:])

                    nc.sync.dma_start(out=bounce[h][:], in_=ctxT_s[:, h * S:(h + 1) * S])
                    nc.gpsimd.collective_compute(
                        "AllGather",
                        mybir.AluOpType.bypass,
                        replica_groups=GROUPS,
                        ins=[bounce[h].opt()],
                        outs=[gath[h].opt()],
                    )

            # ---------- Phase 3: output projection (column shard) ----------
            with (
                tc.tile_pool(name="gs", bufs=3) as gpool,
                tc.tile_pool(name="ob", bufs=2) as opool,
                tc.tile_pool(name="p3", bufs=1, space="PSUM") as p3,
            ):
                NTT = 8  # tok tiles per pass
                for pa in range(2):
                    ops = [p3.tile([128, TB], f32, tag=f"ops{i}", name=f"ops{i}")
                           for i in range(NTT)]
                    for gi, (h, j) in enumerate(
                            (h, j) for h in range(HLOC) for j in range(HLOC)):
                        g = 4 * j + h
                        gs = gpool.tile([128, NTT * 128], bf16, tag="gs")
                        nc.sync.dma_start(
                            out=gs[:],
                            in_=gath[h][j * 128:(j + 1) * 128,
                                        pa * NTT * 128:(pa + 1) * NTT * 128],
                        )
                        for i in range(NTT):
                            mm(ops[i][:], gs[:, i * 128:(i + 1) * 128],
                               wo_s[:, g * TB:(g + 1) * TB],
                               start=(gi == 0), stop=(gi == HT - 1))
                    for i in range(NTT):
                        osb = opool.tile([128, TB], f32, tag="osb")
                        nc.vector.tensor_copy(osb[:], ops[i][:])
                        tt = pa * NTT + i
                        nc.sync.dma_start(out=out_o[tt * 128:(tt + 1) * 128, :],
                                          in_=osb[:])
        stk0.__exit__(None, None, None)
    nc.compile()
    return nc


def kernel(x, wq, wk, wv, wo, cos, sin, mask):
    global LAST_RESULTS
    import ml_dtypes
    from concourse.bass_utils import run_bass_kernel_spmd

    if os.environ.get("BASS_TRACE"):
        # The axon trace path needs antenv.axon_hooks; if this image lacks
        # it, fall back to untraced execution instead of crashing.
        try:
            import antenv.axon_hooks  # noqa: F401
        except ImportError:
            os.environ["BASS_NEVER_TRACE"] = "1"

    bfnp = ml_dtypes.bfloat16
    x = np.asarray(x, np.float32)
    wq = np.asarray(wq, np.float32)
    wk = np.asarray(wk, np.float32)
    wv = np.asarray(wv, np.float32)
    wo = np.asarray(wo, np.float32)
    cos = np.asarray(cos, np.float32)
    sin = np.asarray(sin, np.float32)

    live, mixd, uniq = _analyze_mask(mask)
    n_u = len(uniq)
    mmask = (np.concatenate(uniq, axis=0) if n_u
             else np.zeros((128, TB), np.float32))

    cosE = np.repeat(cos, 2, axis=1).T
    sp = np.repeat(sin, 2, axis=1).copy()
    sp[:, 0::2] *= -1.0
    sinP = sp.T
    pswap = np.zeros((128, 128), np.float32)
    pswap[np.arange(128), np.arange(128) ^ 1] = 1.0
    ident = np.eye(128, dtype=np.float32)

    nc = _build_program(live, mixd, n_u)

    def b(a):
        return np.ascontiguousarray(np.asarray(a).astype(bfnp))

    in_maps = []
    for c in range(N_CORES):
        bb, j = c // 4, c % 4
        in_maps.append({
            "xT": b(x[bb].T),
            "wq": b(wq[:, 512 * j:512 * (j + 1)]),
            "wk": b(wk[:, 128 * j:128 * (j + 1)]),
            "wv": b(wv[:, 128 * j:128 * (j + 1)]),
            "wo": b(wo[:, 512 * j:512 * (j + 1)]),
            "cosE": b(cosE), "sinP": b(sinP), "pswap": b(pswap),
            "ident": b(ident),
            "ones_in": b(np.ones((128, 1), np.float32)),
            "onesk1_in": b(np.ones((1, 128), np.float32)),
            "mmask": b(mmask),
        })

    res = run_bass_kernel_spmd(nc, in_maps, list(range(N_CORES)))
    LAST_RESULTS = res

    out = np.empty((B, S, HID), np.float32)
    for c in range(N_CORES):
        bb, j = c // 4, c % 4
        out[bb, :, 512 * j:512 * (j + 1)] = res.results[c]["o"]
    return out
